# revision 43
# baseline (speedup 1.0000x reference)
"""BitNet Llama attention (B=2, S=2048, H=4096, 32 q-heads / 8 kv-heads, GQA),
distributed over 8 Trainium2 NeuronCores.

Sharding: token-sharded BitLinear QKV projections + activation quantization
(core c owns 512 consecutive global tokens), AllToAll to head-sharded
attention (core c = kv-head c + q-heads 4c..4c+3, full causal triangle —
identical instruction stream on every core, as SPMD requires), AllReduce(max)
for the o-proj activation scales (+ReduceScatter for the own-token slice,
overlapped with quantization), quantize + transpose on the sender, AllToAll
back to token shards, BitLinear o_proj, host concat of row slices.

Perf structure: weights are pre-tiled on the host so every weight-slab DMA is
contiguous; all 128x128 transposes run on the PE (keeps the tensor engine HAM
warm); Q projection runs first so its AllToAll hides under the K/V
projections; o-quant output is transposed on the sending side so the receive
side is pure bulk DMA; Wo slabs prefetch during attention.

BitLinear exactness: weights are ternarized on host and shipped as bf16
{-1,0,1}; activations are quantized on-chip to the int8 grid (magic-number
round-half-even) and stored as bf16 integers; bf16 x bf16 matmuls accumulate
exact integers in fp32 PSUM; per-token dequant scales are applied afterwards.
"""

import math
import os
import sys
from contextlib import ExitStack

import numpy as np
import ml_dtypes

for _p in ("/opt/trn_rl_repo", os.path.expanduser("~/.axon_site/_ro/trn_rl_repo")):
    if os.path.isdir(_p) and _p not in sys.path:
        sys.path.insert(0, _p)

import concourse.bass as bass
import concourse.mybir as mybir
import concourse.tile as tile
from concourse import bacc
from concourse.masks import make_identity

P = 128
H = 4096
DHEAD = 128
NH = 32
NKV = 8
NCORES = 8
MAGIC = 12582912.0  # 1.5 * 2**23: fp32 round-half-even via add/sub
LN2 = float(math.log(2.0))
INV_SQRT_D = float(np.float32(1.0) / np.float32(np.sqrt(np.float32(DHEAD))))
INV127 = float(np.float32(1.0) / np.float32(127.0))

F32 = mybir.dt.float32
BF16 = mybir.dt.bfloat16
MULT = mybir.AluOpType.mult
ADD = mybir.AluOpType.add
SUB = mybir.AluOpType.subtract
MAXOP = mybir.AluOpType.max
COPYF = mybir.ActivationFunctionType.Copy


def build_program(S=2048, B=2, collectives=True):
    """One SPMD program; per-core behavior differs only through input data."""
    T_GLOB = B * S                      # global tokens
    T_OWN = T_GLOB // NCORES            # tokens owned per core
    NT = T_OWN // P                     # own token tiles (4 at S=2048)
    QTB = S // P                        # q tiles per batch (16)
    QT_ALL = B * QTB                    # global token tiles (32)
    HT = H // P                         # hidden tiles (32)
    GF = H // NCORES                    # q-features per head group (512)
    NVT = NKV * DHEAD // 256            # 256-wide V slabs (4)

    # A2A chunk layouts (bf16):
    CH_Q = 4 * P * T_OWN                # [f 4][p 128][t T_OWN]
    CH_K = P * T_OWN                    # [p][t]
    CH_V = NT * P * P                   # [i NT][p][d 128]
    CH_KV = CH_K + CH_V
    CH_O = P * NT * T_OWN               # [p][f NT][t T_OWN]  (pre-transposed)

    nc = bacc.Bacc(
        "TRN2", target_bir_lowering=False, debug=False, num_devices=NCORES
    )
    groups = [list(range(NCORES))]

    x_own = nc.dram_tensor("x_own", [T_OWN, H], F32, kind="ExternalInput")
    wq_t = nc.dram_tensor("wq_t", [HT, P, H], BF16, kind="ExternalInput")
    wk_t = nc.dram_tensor("wk_t", [NKV, P, H], BF16, kind="ExternalInput")
    wv_t = nc.dram_tensor("wv_t", [NVT, P, HT * 256], BF16, kind="ExternalInput")
    wo_t = nc.dram_tensor("wo_t", [16, P, HT * 256], BF16, kind="ExternalInput")
    scal = nc.dram_tensor("scal", [P, 8], F32, kind="ExternalInput")
    cmaskT = nc.dram_tensor("cmaskT", [P, 4 * P], BF16, kind="ExternalInput")
    out_own = nc.dram_tensor("out_own", [T_OWN, H], F32, kind="ExternalOutput")

    with tile.TileContext(nc) as tc, ExitStack() as ctx:
        dram = ctx.enter_context(tc.tile_pool(name="dram", bufs=1, space="DRAM"))
        const = ctx.enter_context(tc.tile_pool(name="const", bufs=1))

        q_in = dram.tile([NCORES, CH_Q], BF16, allow_tmpbuf=True)
        q_out = dram.tile([NCORES, CH_Q], BF16, allow_tmpbuf=True)
        kv_in = dram.tile([NCORES, CH_KV], BF16, allow_tmpbuf=True)
        kv_out = dram.tile([NCORES, CH_KV], BF16, allow_tmpbuf=True)
        TBW = NT * (T_OWN // B) + 4         # o-chunk row: data + amax hi/lo
        CH_OB = P * TBW                     # o-chunk elems per batch slot
        pamax_db = [dram.tile([S], F32, name=f"pamax{b}") for b in range(B)]
        amax_all_db = [dram.tile([S], F32, name=f"amaxall{b}")
                       for b in range(B)]
        xoq_in_b = [dram.tile([NCORES, CH_OB], BF16, allow_tmpbuf=True,
                              name=f"xoqin{b}") for b in range(B)]
        xoq_out_b = [dram.tile([NCORES, CH_OB], BF16, allow_tmpbuf=True,
                               name=f"xoqout{b}") for b in range(B)]

        ident = const.tile([P, P], BF16)
        make_identity(nc, ident)
        cmask_sb = const.tile([P, 4 * P], BF16)
        nc.sync.dma_start(cmask_sb[:], cmaskT[:, :])
        scal_sb = const.tile([P, 8], F32)
        nc.sync.dma_start(scal_sb[:], scal[:, :])

        # Pool stack (LIFO close order): pat (attention operands), gw (Wo
        # slabs), pos (o_slice) live to the end; pxq (xq) closes after B;
        # pxo (received xoq) opens/closes inside the tail block.
        pat_cm = tc.tile_pool(name="pat", bufs=1)
        pat = pat_cm.__enter__()
        gw_cm = tc.tile_pool(name="gw", bufs=2)
        gw = gw_cm.__enter__()
        pos_cm = tc.tile_pool(name="pos", bufs=1)
        pos = pos_cm.__enter__()
        pxq_cm = tc.tile_pool(name="pxq", bufs=1)
        pxq = pxq_cm.__enter__()
        xqT = pxq.tile([P, HT, T_OWN], BF16)           # quantized x, transposed
        dq_cols = pxq.tile([P, NT], F32)               # amax_clip/127 per own token

        # ---- Phase A: load x, quantize to int8 grid, transpose on PE ----
        with tc.tile_pool(name="qwork", bufs=2) as qwork, \
             tc.tile_pool(name="psa", bufs=4, space="PSUM") as psa:
            for ti in range(NT):
                x_t = qwork.tile([P, H], F32, tag="x")
                nc.sync.dma_start(x_t[:], x_own[ti * P:(ti + 1) * P, :])
                amax = qwork.tile([P, 1], F32, tag="amax")
                nc.vector.tensor_reduce(
                    amax[:], x_t[:], mybir.AxisListType.X, MAXOP,
                    apply_absolute_value=True,
                )
                amax_c = qwork.tile([P, 1], F32, tag="amaxc")
                nc.vector.tensor_scalar(amax_c[:], amax[:], 1e-5, None, MAXOP)
                inv = qwork.tile([P, 1], F32, tag="inv")
                nc.vector.reciprocal(inv[:], amax_c[:])
                a_col = qwork.tile([P, 1], F32, tag="acol")
                nc.vector.tensor_scalar(a_col[:], inv[:], 127.0, None, MULT)
                nc.vector.tensor_scalar(
                    dq_cols[:, ti:ti + 1], amax_c[:], INV127, None, MULT
                )
                nc.scalar.activation(
                    x_t[:], x_t[:], COPYF, bias=MAGIC, scale=a_col[:]
                )
                xq = qwork.tile([P, H], BF16, tag="xq")
                nc.vector.tensor_scalar(xq[:], x_t[:], MAGIC, None, SUB)
                for hi in range(HT):
                    pst = psa.tile([P, P], BF16, tag="pt")
                    nc.tensor.transpose(
                        pst[:], xq[:, hi * P:(hi + 1) * P], ident[:]
                    )
                    dst = xqT[:, hi, ti * P:(ti + 1) * P]
                    if hi % 2 == 0:
                        nc.scalar.copy(dst, pst[:])
                    else:
                        nc.vector.tensor_copy(dst, pst[:])

        # ---- Phase A2: broadcast per-token dequant rows across partitions ----
        bcast_q = pxq.tile([P, T_OWN], F32)
        bcast_k = pxq.tile([P, T_OWN], F32)
        with tc.tile_pool(name="bwork", bufs=1) as bwork, \
             tc.tile_pool(name="psb", bufs=2, space="PSUM") as psb:
            dq_row = bwork.tile([1, T_OWN], F32)
            for ti in range(NT):
                nc.sync.dma_start(
                    dq_row[0:1, ti * P:(ti + 1) * P], dq_cols[:, ti:ti + 1]
                )
            ones_row = bwork.tile([1, P], F32)
            nc.vector.memset(ones_row[:], 1.0)
            srow_q = bwork.tile([1, T_OWN], F32)
            nc.vector.tensor_scalar(
                srow_q[:], dq_row[:], scal_sb[0:1, 0:1], INV_SQRT_D, MULT, MULT
            )
            srow_k = bwork.tile([1, T_OWN], F32)
            nc.vector.tensor_scalar(
                srow_k[:], dq_row[:], scal_sb[0:1, 1:2], None, MULT
            )
            for src, dst in ((srow_q, bcast_q), (srow_k, bcast_k)):
                ps = psb.tile([P, T_OWN], F32, tag="b")
                nc.tensor.matmul(ps[:], ones_row[:], src[:], start=True, stop=True)
                nc.vector.tensor_copy(dst[:], ps[:])

        # ---- Phase B: QKV projections (token-sharded) -> A2A chunks ----
        q_in_r = q_in.rearrange("r (f p t) -> r f p t", f=4, p=P)
        k_in_r = kv_in[:, 0:CH_K].rearrange("r (p t) -> r p t", p=P)
        v_in_r = kv_in[:, CH_K:CH_KV].rearrange("r (i p d) -> r i p d", i=NT, p=P)

        # attention operands: q assembly overlaps K/V proj
        qT_grp = pat.tile([P, 4, T_GLOB], BF16)
        kT_full = pat.tile([P, T_GLOB], BF16)
        v_full = pat.tile([P, QT_ALL, 132], BF16)

        with tc.tile_pool(name="wslab", bufs=3) as wslab, \
             tc.tile_pool(name="pevac", bufs=3) as pevac, \
             tc.tile_pool(name="psp", bufs=3, space="PSUM") as psp:
            for dj in range(HT):                    # q feature tiles
                wsl = wslab.tile([P, HT, P], BF16, tag="wq", bufs=2)
                nc.sync.dma_start(wsl[:], wq_t[dj, :, :])
                ps = psp.tile([P, T_OWN], F32, tag="p")
                for hi in range(HT):
                    nc.tensor.matmul(
                        ps[:], wsl[:, hi, :], xqT[:, hi, :],
                        start=(hi == 0), stop=(hi == HT - 1),
                    )
                ev = pevac.tile([P, T_OWN], BF16, tag="e")
                nc.vector.tensor_tensor(ev[:], ps[:], bcast_q[:], MULT)
                nc.sync.dma_start(q_in_r[dj // 4, dj % 4, :, :], ev[:])

            if collectives:
                nc.gpsimd.collective_compute(
                    "AllToAll", mybir.AluOpType.bypass, replica_groups=groups,
                    ins=[q_in[:, :].opt()], outs=[q_out[:, :].opt()],
                )
            else:
                nc.sync.dma_start(q_out[:, :], q_in[:, :])

            # q operand assembly (overlaps K/V projection below)
            q_out_r = q_out.rearrange("r (f p t) -> r p f t", f=4, p=P)
            for s in range(NCORES):
                nc.sync.dma_start(
                    qT_grp[:, :, s * T_OWN:(s + 1) * T_OWN], q_out_r[s, :, :, :]
                )

            for dj in range(NKV):                   # kv-head feature tiles
                wsl = wslab.tile([P, HT, P], BF16, tag="wq", bufs=2)
                nc.sync.dma_start(wsl[:], wk_t[dj, :, :])
                ps = psp.tile([P, T_OWN], F32, tag="p")
                for hi in range(HT):
                    nc.tensor.matmul(
                        ps[:], wsl[:, hi, :], xqT[:, hi, :],
                        start=(hi == 0), stop=(hi == HT - 1),
                    )
                ev = pevac.tile([P, T_OWN], BF16, tag="e")
                nc.vector.tensor_tensor(ev[:], ps[:], bcast_k[:], MULT)
                nc.sync.dma_start(k_in_r[dj, :, :], ev[:])
            for vi in range(NVT):                   # v natural layout
                wsl = wslab.tile([P, HT, 256], BF16, tag="wv", bufs=2)
                nc.sync.dma_start(wsl[:], wv_t[vi, :, :])
                for ti in range(NT):
                    ps = psp.tile([P, 256], F32, tag="pv")
                    for hi in range(HT):
                        nc.tensor.matmul(
                            ps[:], xqT[:, hi, ti * P:(ti + 1) * P], wsl[:, hi, :],
                            start=(hi == 0), stop=(hi == HT - 1),
                        )
                    sv = pevac.tile([P, 1], F32, tag="sv")
                    nc.vector.tensor_scalar(
                        sv[:], dq_cols[:, ti:ti + 1], scal_sb[:, 2:3], None, MULT
                    )
                    ev = pevac.tile([P, 256], BF16, tag="ev")
                    nc.scalar.mul(ev[:], ps[:], sv[:])
                    for sub in range(2):
                        nc.sync.dma_start(
                            v_in_r[vi * 2 + sub, ti, :, :],
                            ev[:, sub * P:(sub + 1) * P],
                        )

        pxq_cm.__exit__(None, None, None)

        # ---- Phase C: AllToAll k/v to head shards ----
        if collectives:
            nc.gpsimd.collective_compute(
                "AllToAll", mybir.AluOpType.bypass, replica_groups=groups,
                ins=[kv_in[:, :].opt()], outs=[kv_out[:, :].opt()],
            )
        else:
            nc.sync.dma_start(kv_out[:, :], kv_in[:, :])

        # ---- Phase D: assemble k/v attention operands ----
        k_out_r = kv_out[:, 0:CH_K].rearrange("r (p t) -> r p t", p=P)
        v_out_r = kv_out[:, CH_K:CH_KV].rearrange("r (i p d) -> r p i d", i=NT, p=P)
        nc.vector.memset(v_full[:], 1.0)  # column 128 = denominator ones
        for s in range(NCORES):
            nc.sync.dma_start(
                kT_full[:, s * T_OWN:(s + 1) * T_OWN], k_out_r[s, :, :]
            )
            nc.sync.dma_start(
                v_full[:, s * NT:(s + 1) * NT, 0:P], v_out_r[s, :, :, :]
            )

        # ---- prefetch Wo slabs for G(b0); G(b1) reloads its own set ----
        wo_tiles = []
        for nj in range(16):
            wsl = gw.tile([P, HT, 256], BF16, tag="wo")
            nc.sync.dma_start(wsl[:], wo_t[nj, :, :])
            wo_tiles.append(wsl)

        def load_wo_slab(nj):
            wsl = gw.tile([P, HT, 256], BF16, tag="wo")
            nc.sync.dma_start(wsl[:], wo_t[nj, :, :])
            return wsl

        # ---- Phases E/E2/F/G: batch-pipelined attention -> o_proj ----
        # o-proj token ownership remap: core r o-projects tokens
        # r*256..(r+1)*256 of EACH batch, so each batch's ReduceScatter
        # shard is exactly that core's dqo slice and per-batch o_proj work
        # is balanced. Batch 0's amax-reduce/quantize/A2A/o_proj pipeline
        # under batch 1's attention.
        TB = T_OWN // B                       # own o-tokens per batch (256)
        TBT = TB // P                         # tiles of those (2)
        OWN_B = TB

        o_slices = [pos.tile([P, QTB, GF], BF16, tag="osl", bufs=2,
                             name=f"osl{b}") for b in range(B)]
        pamax_sbs = [pos.tile([P, QTB], F32, tag="pam", bufs=2,
                              name=f"pam{b}") for b in range(B)]
        xoq_in_rb = [x.rearrange("r (p w) -> r p w", p=P) for x in xoq_in_b]
        xoq_out_rb = [x.rearrange("r (p w) -> r p w", p=P) for x in xoq_out_b]

        def emit_attention(b, qb_lo, qb_hi, att, pss, pso):
            o_slice_b = o_slices[b]
            pamax_b = pamax_sbs[b]
            for qb in range(qb_lo, qb_hi):
                qt = b * QTB + qb
                # Heads 0-2 packed in one PSUM bank: head 0's j=0 start=True
                # clears the whole bank's has_written bits; heads 1-2 then
                # overwrite-on-first-write / accumulate-after (start=False).
                oa = pso.tile([P, 3, 132], F32, tag="oa")
                ob = pso.tile([P, 132], F32, tag="ob")
                po = [oa[:, 0, :], oa[:, 1, :], oa[:, 2, :], ob[:]]
                pt_all = att.tile([P, QTB, 4 * P], BF16, tag="pt", bufs=2)
                for j0 in range(0, qb + 1, 2):
                    npair = min(2, qb + 1 - j0)
                    ps = pss.tile([P, 2, 4 * P], F32, tag="s")
                    for u in range(npair):
                        kt = b * QTB + j0 + u
                        nc.tensor.matmul(
                            ps[:, u, :],
                            kT_full[:, kt * P:(kt + 1) * P],
                            qT_grp[:, :, qt * P:(qt + 1) * P],
                            start=True, stop=True,
                        )
                    nc.scalar.activation(
                        pt_all[:, j0:j0 + npair, :], ps[:, 0:npair, :],
                        mybir.ActivationFunctionType.Exp, scale=LN2,
                    )
                    if j0 + npair - 1 == qb:
                        nc.vector.tensor_tensor(
                            pt_all[:, qb, :], pt_all[:, qb, :],
                            cmask_sb[:], MULT,
                        )
                    for u in range(npair):
                        j = j0 + u
                        kt = b * QTB + j
                        for hl in range(4):
                            nc.tensor.matmul(
                                po[hl][0:P, 0:129],
                                pt_all[:, j, hl * P:(hl + 1) * P],
                                v_full[:, kt, 0:129],
                                start=(j == 0 and hl in (0, 3)),
                                stop=(j == qb),
                                skip_group_check=(hl in (1, 2)),
                            )
                for hl in range(4):
                    den = att.tile([P, 1], F32, tag="den")
                    nc.vector.reciprocal(den[:], po[hl][0:P, 128:129])
                    nc.scalar.mul(
                        o_slice_b[:, qb, hl * P:(hl + 1) * P],
                        po[hl][0:P, 0:P], den[:],
                    )
                nc.vector.tensor_reduce(
                    pamax_b[:, qb:qb + 1], o_slice_b[:, qb, :],
                    mybir.AxisListType.X, MAXOP, apply_absolute_value=True,
                )
                nc.sync.dma_start(
                    pamax_db[b][qb * P:(qb + 1) * P], pamax_b[:, qb:qb + 1]
                )

        def emit_amax_collectives(b):
            if collectives:
                nc.gpsimd.collective_compute(
                    "AllReduce", MAXOP, replica_groups=groups,
                    ins=[pamax_db[b][:].opt()], outs=[amax_all_db[b][:].opt()],
                )
            else:
                nc.sync.dma_start(amax_all_db[b][:], pamax_db[b][:])

        def emit_e2(b, oq, ost, pse2, pe_transpose):
            # quantize + transpose o for batch b, then A2A it out.
            # Chunk row layout per partition: [f 4][t 256] data + [amax hi 2]
            # [amax lo 2] (per-token clipped amax as exact-ish bf16 hi/lo).
            a_all = oq.tile([P, QTB], F32, tag="aall", bufs=2)
            for qb in range(QTB):
                nc.sync.dma_start(
                    a_all[:, qb:qb + 1], amax_all_db[b][qb * P:(qb + 1) * P]
                )
            am_c = oq.tile([P, QTB], F32, tag="amc", bufs=2)
            nc.vector.tensor_scalar(am_c[:], a_all[:], 1e-5, None, MAXOP)
            inv_all = oq.tile([P, QTB], F32, tag="oinv", bufs=2)
            nc.vector.reciprocal(inv_all[:], am_c[:])
            acol_all = oq.tile([P, QTB], F32, tag="oacol", bufs=2)
            nc.vector.tensor_scalar(acol_all[:], inv_all[:], 127.0, None, MULT)
            for r in range(NCORES):
                stage = ost.tile([P, TBW], BF16, tag="st")
                st_d = stage[:, 0:NT * TB].rearrange("p (f t) -> p f t", f=NT)
                for sub in range(TBT):
                    qb = r * TBT + sub
                    xr = oq.tile([P, GF], F32, tag="oxr")
                    nc.vector.tensor_scalar(
                        xr[:], o_slices[b][:, qb, :], acol_all[:, qb:qb + 1],
                        MAGIC, MULT, ADD,
                    )
                    xq = oq.tile([P, GF], BF16, tag="oxq")
                    nc.vector.tensor_scalar(xq[:], xr[:], MAGIC, None, SUB)
                    for fi in range(NT):
                        dst = st_d[:, fi, sub * P:(sub + 1) * P]
                        if pe_transpose:
                            pst = pse2.tile([P, P], BF16, tag="pt")
                            nc.tensor.transpose(
                                pst[:], xq[:, fi * P:(fi + 1) * P], ident[:]
                            )
                            nc.vector.tensor_copy(dst, pst[:])
                        else:
                            nc.sync.dma_start_transpose(
                                dst, xq[:, fi * P:(fi + 1) * P]
                            )
                # amax hi/lo for this destination's two token tiles
                amh = stage[:, NT * TB:NT * TB + TBT]
                aml = stage[:, NT * TB + TBT:TBW]
                nc.vector.tensor_copy(
                    amh, am_c[:, r * TBT:(r + 1) * TBT]
                )
                res = oq.tile([P, TBT], F32, tag="ares")
                nc.vector.tensor_tensor(
                    res[:], am_c[:, r * TBT:(r + 1) * TBT], amh, SUB
                )
                nc.vector.tensor_copy(aml, res[:])
                nc.sync.dma_start(xoq_in_rb[b][r, :, :], stage[:])
            if collectives:
                nc.gpsimd.collective_compute(
                    "AllToAll", mybir.AluOpType.bypass, replica_groups=groups,
                    ins=[xoq_in_b[b][:, :].opt()], outs=[xoq_out_b[b][:, :].opt()],
                )
            else:
                nc.sync.dma_start(xoq_out_b[b][:, :], xoq_in_b[b][:, :])

        def emit_fg_head(b, gev):
            # load received (pre-transposed) xoq + per-token dq scales
            xoqT = pxo.tile([P, HT, TB], BF16, tag="xoqT")
            for s in range(NCORES):
                nc.sync.dma_start(
                    xoqT[:, s * NT:(s + 1) * NT, :],
                    xoq_out_rb[b][s, :, 0:NT * TB].rearrange(
                        "p (f t) -> p f t", f=NT
                    ),
                )
            amh = gev.tile([P, TBT], BF16, tag="amh", bufs=2)
            nc.sync.dma_start(amh[:], xoq_out_rb[b][0, :, NT * TB:NT * TB + TBT])
            aml = gev.tile([P, TBT], BF16, tag="aml", bufs=2)
            nc.sync.dma_start(aml[:], xoq_out_rb[b][0, :, NT * TB + TBT:TBW])
            dqo = gev.tile([P, TBT], F32, tag="dqo", bufs=2)
            nc.vector.tensor_tensor(dqo[:], amh[:], aml[:], ADD)
            nc.vector.tensor_scalar(
                dqo[:], dqo[:], scal_sb[:, 3:4], INV127, MULT, MULT
            )
            return xoqT, dqo

        def emit_fg_body(b, gev, psg, xoqT, dqo, nj_lo, nj_hi):
            for nj in range(nj_lo, nj_hi):
                wsl = wo_tiles[nj] if b == 0 else load_wo_slab(nj)
                for ti in range(TBT):
                    ps = psg.tile([P, 256], F32, tag="g")
                    for hi in range(HT):
                        nc.tensor.matmul(
                            ps[:], xoqT[:, hi, ti * P:(ti + 1) * P],
                            wsl[:, hi, :],
                            start=(hi == 0), stop=(hi == HT - 1),
                        )
                    ev = gev.tile([P, 256], F32, tag="ge")
                    nc.scalar.mul(ev[:], ps[:], dqo[:, ti:ti + 1])
                    nc.sync.dma_start(
                        out_own[b * OWN_B + ti * P:b * OWN_B + (ti + 1) * P,
                                nj * 256:(nj + 1) * 256], ev[:]
                    )

        with tc.tile_pool(name="gev", bufs=3) as gev, \
             tc.tile_pool(name="oq", bufs=4) as oq, \
             tc.tile_pool(name="ost", bufs=2) as ost, \
             tc.tile_pool(name="pse2", bufs=2, space="PSUM") as pse2:
            with tc.tile_pool(name="att", bufs=4) as att, \
                 tc.tile_pool(name="pss", bufs=2, space="PSUM") as pss, \
                 tc.tile_pool(name="pso", bufs=1, space="PSUM") as pso:
                emit_attention(0, 0, QTB, att, pss, pso)
                emit_amax_collectives(0)
                emit_attention(1, 0, 10, att, pss, pso)
                emit_e2(0, oq, ost, pse2, pe_transpose=True)  # under E(b1)
                emit_attention(1, 10, QTB, att, pss, pso)
            emit_amax_collectives(1)
            pxo_cm = tc.tile_pool(name="pxo", bufs=2)
            pxo = pxo_cm.__enter__()
            with tc.tile_pool(name="psg", bufs=2, space="PSUM") as psg:
                x0, d0 = emit_fg_head(0, gev)
                emit_fg_body(0, gev, psg, x0, d0, 0, 4)
                emit_e2(1, oq, ost, pse2, pe_transpose=True)  # under G(b0)
                emit_fg_body(0, gev, psg, x0, d0, 4, 16)
                x1, d1 = emit_fg_head(1, gev)
                emit_fg_body(1, gev, psg, x1, d1, 0, 16)
            pxo_cm.__exit__(None, None, None)

        pos_cm.__exit__(None, None, None)
        gw_cm.__exit__(None, None, None)
        pat_cm.__exit__(None, None, None)

    nc.compile()
    return nc


def _ternarize(W):
    ws = np.float32(max(np.mean(np.abs(W), dtype=np.float32), np.float32(1e-5)))
    t = np.clip(np.round(W / ws), -1.0, 1.0).astype(np.float32)
    return t, ws


def prepare_inputs(hidden_states, Wq, Wk, Wv, Wo, S=2048, B=2):
    bf16 = ml_dtypes.bfloat16
    T_GLOB = B * S
    T_OWN = T_GLOB // NCORES
    HT = H // P
    x = np.ascontiguousarray(
        np.asarray(hidden_states, dtype=np.float32).reshape(T_GLOB, H)
    )
    tq, wqs = _ternarize(np.asarray(Wq, dtype=np.float32))
    tk, wks = _ternarize(np.asarray(Wk, dtype=np.float32))
    tv, wvs = _ternarize(np.asarray(Wv, dtype=np.float32))
    to, wos = _ternarize(np.asarray(Wo, dtype=np.float32))

    def _tile_w(tW, width):
        # tW: [out, hidden] ternary. Slab layout: [slab, p, hi, c] where
        # element = tW.T[hi*128+p, slab*width+c], contiguous per slab.
        wT = np.ascontiguousarray(tW.T)                      # [H, out]
        nslab = wT.shape[1] // width
        t = wT.reshape(HT, P, nslab, width).transpose(2, 1, 0, 3)
        return np.ascontiguousarray(t.reshape(nslab, P, HT * width)).astype(bf16)

    wq_t = _tile_w(tq, P)
    wk_t = _tile_w(tk, P)
    wv_t = _tile_w(tv, 256)
    wo_t = _tile_w(to, 256)
    scal = np.zeros((P, 8), np.float32)
    scal[:, 0] = wqs
    scal[:, 1] = wks
    scal[:, 2] = wvs
    scal[:, 3] = wos
    kk, qq = np.meshgrid(np.arange(P), np.arange(P), indexing="ij")
    cmaskT = np.tile((kk <= qq).astype(np.float32).astype(bf16), (1, 4))
    shared = dict(wq_t=wq_t, wk_t=wk_t, wv_t=wv_t, wo_t=wo_t, scal=scal,
                  cmaskT=cmaskT)
    return [
        dict(x_own=np.ascontiguousarray(x[c * T_OWN:(c + 1) * T_OWN]), **shared)
        for c in range(NCORES)
    ]


_PROGRAM_CACHE = {}


def kernel(hidden_states, attention_mask, Wq, Wk, Wv, Wo):
    from concourse.bass_utils import run_bass_kernel_spmd

    B, S, _ = hidden_states.shape
    key = (B, S)
    if key not in _PROGRAM_CACHE:
        _PROGRAM_CACHE[key] = build_program(S=S, B=B)
    nc = _PROGRAM_CACHE[key]
    in_maps = prepare_inputs(hidden_states, Wq, Wk, Wv, Wo, S=S, B=B)
    res = run_bass_kernel_spmd(
        nc, in_maps, core_ids=list(range(NCORES)),
        trace=bool(int(os.environ.get("KERNEL_TRACE", "0"))),
    )
    # out_own rows [b*(S//NCORES) + t] hold batch b, seq pos c*(S//NCORES)+t
    big = np.stack([r["out_own"] for r in res.results])   # [NCORES, T_OWN, H]
    sb = S // NCORES
    out = np.stack(
        [big[:, b * sb:(b + 1) * sb, :].reshape(S, H) for b in range(B)]
    )
    kernel.last_results = res
    return np.ascontiguousarray(out).astype(np.float32)
